# revision 13
# baseline (speedup 1.0000x reference)
"""Trainium2 Bass kernel for ExpanderLinear: out = x @ (W * mask).T

Shapes (hardcoded): x [8192, 4096] f32, weight [4096, 4096] f32,
mask [4096, 4096] f32 -> out [8192, 4096] f32.

Strategy: tensor-parallel over output features across 8 cores. The host
pre-marshals operands (like GEMM pre-packing): wm = (W*mask)*32
premultiplied, transposed, and split along the contraction dim:
  - rows 0..3583  -> bf16   (28 of 32 contraction chunks)
  - rows 3584..4095 -> fp8e4m3, computed with DoubleRow matmuls
    (2 contraction chunks of 256 per instruction, 2 MACs/cell/cycle)
x is transposed and split the same way (bf16 + fp8). The *32 weight
scale (exact in bf16, keeps fp8 weights out of the subnormal range) is
undone by the PSUM-drain copy (tensor_scalar_mul 1/32). Measured absmax
error vs the f64 reference: 1.80e-2 of scale (tolerance 2e-2); the
bf16-only variant measures 2.45e-3.

All host arrays are pre-packed into the exact SBUF tile layouts so
every DMA moves one fully contiguous block (no strided gathers):
  xTp   [NC][KGB][P][KCG][1024] bf16   (chunks 1..7; slot 0 unused)
  x0Tp  [2][KGB][P][KCG][512]   bf16   (chunk 0, per batch half)
  x8Tp  [NC][P][KP][2][1024]    fp8    (chunks 1..7)
  x80Tp [2][P][KP][2][512]      fp8    (chunk 0)
  wmTp  [KGB][P][KCG][512]      bf16
  wm8Tp [P][KP][2][512]         fp8
  outTp [OT][NC][2][P][512]     f32    (drain DMA writes contiguous)

Per-core device kernel:
  - PE warmup matmuls on a memset tile run during the initial DMA wait
    so the HAM clock gate is at 2.4 GHz when data lands.
  - weights persist in SBUF (3.5 MB bf16 + 0.25 MB fp8), loaded
    interleaved with the first chunk's x loads (chunk 0 uses 512-col
    half tiles so the first matmul's dependencies are only ~1 MB).
  - loop over 8 batch chunks of 1024, each split into two 512-wide
    matmul halves: 4 psum banks per half; per oc 28 bf16 matmuls + 2
    fp8 DoubleRow matmuls accumulate. The 2 DR matmuls are interleaved
    with the last bf16 matmuls (DR,bf16 alternation) so the DR
    LDWEIGHTS (non-FWL, ~107 ns) hides under a 213 ns bf16 matmul
    instead of serializing. Each oc's drain is emitted right after its
    stop matmul so the psum bank frees early. Prefetch for chunk c+1 is
    spread through c's second half for ~25 us of DMA lead time.
"""

import ml_dtypes
import numpy as np

import concourse.bass as bass
import concourse.mybir as mybir
import concourse.tile as tile
from concourse import bacc
from concourse.bass_utils import run_bass_kernel_spmd

P = 128
D_IN = 4096
D_OUT = 4096
BATCH = 8192
N_CORES = 8
O_PER_CORE = D_OUT // N_CORES  # 512
OT = O_PER_CORE // P  # 4 output partition tiles
KCG = 4  # contraction chunks (of 128) per bf16 group
K8_PAIRS = 2  # fp8 DoubleRow matmuls per oc (each covers 256 of K)
K8 = K8_PAIRS * 2 * P  # 512 contraction rows in fp8
KB = D_IN - K8  # 3584 contraction rows in bf16
KGB = KB // (KCG * P)  # 7 bf16 groups
NIC = KB // P  # 28 bf16 contraction chunks
BC_DMA = 1024  # batch columns per DMA tile (2 KB bf16 lines)
N_CHUNK = BATCH // BC_DMA  # 8
BN = 512  # matmul free dim (one psum bank)
N_WARM = 56  # PE warmup matmuls (N=128)
WSCALE = 32.0  # host weight pre-scale, undone in the drain

F32 = mybir.dt.float32
BF16 = mybir.dt.bfloat16
F8 = mybir.dt.float8e4
DR = mybir.MatmulPerfMode.DoubleRow


def build_nc():
    nc = bacc.Bacc("TRN2", target_bir_lowering=False, debug=False, num_devices=N_CORES)

    xT_d = nc.dram_tensor("xTp", [N_CHUNK * KGB * P, KCG, BC_DMA], BF16, kind="ExternalInput")
    x0T_d = nc.dram_tensor("x0Tp", [2 * KGB * P, KCG, BN], BF16, kind="ExternalInput")
    x8T_d = nc.dram_tensor("x8Tp", [N_CHUNK * P, K8_PAIRS, 2, BC_DMA], F8, kind="ExternalInput")
    x80T_d = nc.dram_tensor("x80Tp", [2 * P, K8_PAIRS, 2, BN], F8, kind="ExternalInput")
    wmT_d = nc.dram_tensor("wmTp", [KGB * P, KCG, O_PER_CORE], BF16, kind="ExternalInput")
    wm8T_d = nc.dram_tensor("wm8Tp", [P, K8_PAIRS, 2, O_PER_CORE], F8, kind="ExternalInput")
    outT_d = nc.dram_tensor("outTp", [OT * N_CHUNK * 2 * P, BN], F32, kind="ExternalOutput")

    with tile.TileContext(nc) as tc:
        with (
            tc.tile_pool(name="persist", bufs=1) as persist,
            tc.tile_pool(name="xs", bufs=16) as xspool,
            tc.tile_pool(name="outp", bufs=8) as outp,
            tc.tile_pool(name="mpsum", bufs=8, space="PSUM") as mpsum,
        ):
            # --- PE warmup: emitted first so the tensor queue starts on
            # them while the first DMAs are in flight ---
            wtile = persist.tile([P, P], BF16, name="warm_in")
            nc.gpsimd.memset(wtile, 0)
            wpsum = mpsum.tile([P, BN], F32, name="warm_ps", tag="ps")
            for _ in range(N_WARM):
                nc.tensor.matmul(
                    wpsum[:, 0:P], wtile, wtile, start=True, stop=True
                )

            # --- weight loads, interleaved with the first chunk's x ---
            wm_g = []

            def emit_wm_group(g):
                wm = persist.tile([P, KCG, O_PER_CORE], BF16, name=f"wmT{g}")
                nc.sync.dma_start(wm, wmT_d[g * P : (g + 1) * P])
                wm_g.append(wm)

            def emit_x_group(ch, g):
                """bf16 x group tile, full 1024 cols (chunks 1..7)."""
                xs = xspool.tile([P, KCG, BC_DMA], BF16, tag="xs", name="xs")
                r0 = (ch * KGB + g) * P
                nc.sync.dma_start(xs, xT_d[r0 : r0 + P])
                return xs

            def emit_x0_group(h, g):
                """bf16 x group tile for chunk 0, 512-col half."""
                xs = xspool.tile([P, KCG, BN], BF16, tag="xs", name="xs")
                r0 = (h * KGB + g) * P
                nc.sync.dma_start(xs, x0T_d[r0 : r0 + P])
                return xs

            def emit_x8(ch):
                """fp8 x tile [P, pairs, 2, 1024] in DoubleRow pairing."""
                xs = xspool.tile([P, K8_PAIRS, 2, BC_DMA], F8, tag="xs", name="xs8")
                nc.sync.dma_start(xs, x8T_d[ch * P : (ch + 1) * P])
                return xs

            def emit_x80(h):
                xs = xspool.tile([P, K8_PAIRS, 2, BN], F8, tag="xs", name="xs8")
                nc.sync.dma_start(xs, x80T_d[h * P : (h + 1) * P])
                return xs

            x0 = {}
            for g in range(KGB):
                emit_wm_group(g)
                x0[(0, g)] = emit_x0_group(0, g)
            wm8 = persist.tile([P, K8_PAIRS, 2, O_PER_CORE], F8, name="wm8T")
            nc.sync.dma_start(wm8, wm8T_d[0:P])
            x0[(0, KGB)] = emit_x80(0)
            for g in range(KGB):
                x0[(1, g)] = emit_x0_group(1, g)
            x0[(1, KGB)] = emit_x80(1)

            def lhsT(ic, oc):
                return wm_g[ic // KCG][:, ic % KCG, oc * P : (oc + 1) * P]

            def lhsT8(kp, oc):
                return wm8[:, kp, :, oc * P : (oc + 1) * P]

            def drain(psum, oc, ch, h):
                ob = outp.tile([P, BN], F32)
                nc.vector.tensor_scalar_mul(ob, psum, 1.0 / WSCALE)
                r0 = ((oc * N_CHUNK + ch) * 2 + h) * P
                nc.sync.dma_start(outT_d[r0 : r0 + P], ob)

            def emit_chunk_reuse(ch, xs_g):
                """Chunks 1..6: whole-chunk emission; each stationary
                weight tile feeds two back-to-back matmuls (h0 then h1,
                8 psum banks), recovering most of the per-instruction
                weight-swap cost. DR matmuls stay strictly alternated
                with bf16 ones; drains are DVE-only, emitted right
                after each (oc, h) stop so banks free early."""
                def rhs(g, k, h):
                    return xs_g[g][:, k, h * BN : (h + 1) * BN]

                def rhs8(kp, h):
                    return xs_g[KGB][:, kp, :, h * BN : (h + 1) * BN]

                psums = [
                    [
                        mpsum.tile([P, BN], F32, name=f"ps{oc}h{h}", tag="ps")
                        for h in range(2)
                    ]
                    for oc in range(OT)
                ]
                prefetch = []
                for g in range(KGB):
                    kk = KCG if g < KGB - 1 else KCG - 2
                    for k in range(kk):
                        ic = g * KCG + k
                        for oc in range(OT):
                            for h in range(2):
                                nc.tensor.matmul(
                                    psums[oc][h],
                                    lhsT(ic, oc),
                                    rhs(g, k, h),
                                    start=(ic == 0),
                                    stop=False,
                                )
                        # spread prefetch evenly across the chunk
                        # (~200 GB/s instantaneous) to cut SBUF-port
                        # contention with the matmul rhs stream
                        if ch + 1 < N_CHUNK and ic in (2, 5, 8, 11, 14, 17, 20):
                            prefetch.append(emit_x_group(ch + 1, (ic - 2) // 3))
                for oc in range(OT):
                    for h in range(2):
                        nc.tensor.matmul(
                            psums[oc][h], lhsT8(0, oc), rhs8(0, h),
                            start=False, stop=False, perf_mode=DR,
                        )
                        nc.tensor.matmul(
                            psums[oc][h], lhsT(NIC - 2, oc),
                            rhs(KGB - 1, KCG - 2, h),
                            start=False, stop=False,
                        )
                if ch + 1 < N_CHUNK:
                    prefetch.append(emit_x8(ch + 1))
                for oc in range(OT):
                    for h in range(2):
                        nc.tensor.matmul(
                            psums[oc][h], lhsT8(1, oc), rhs8(1, h),
                            start=False, stop=False, perf_mode=DR,
                        )
                        nc.tensor.matmul(
                            psums[oc][h], lhsT(NIC - 1, oc),
                            rhs(KGB - 1, KCG - 1, h),
                            start=False, stop=True,
                        )
                        drain(psums[oc][h], oc, ch, h)
                return prefetch

            # --- main loop over batch chunks ---
            pending = None
            for ch in range(N_CHUNK):
                if 0 < ch < N_CHUNK - 1:
                    pending = emit_chunk_reuse(ch, pending)
                    continue
                xs_g = pending
                for h in range(2):
                    def rhs(g, k):
                        if ch == 0:
                            return x0[(h, g)][:, k, :]
                        return xs_g[g][:, k, h * BN : (h + 1) * BN]

                    def rhs8(kp):
                        if ch == 0:
                            return x0[(h, KGB)][:, kp, :, :]
                        return xs_g[KGB][:, kp, :, h * BN : (h + 1) * BN]

                    last = ch == N_CHUNK - 1 and h == 1
                    psums = [
                        mpsum.tile([P, BN], F32, name=f"ps{oc}", tag="ps")
                        for oc in range(OT)
                    ]

                    def emit_oc_mms(oc):
                        # oc-major (last half): DR matmuls interleaved
                        # between bf16 ones so their LDWEIGHTS hide.
                        for ic in range(NIC - 2):
                            nc.tensor.matmul(
                                psums[oc],
                                lhsT(ic, oc),
                                rhs(ic // KCG, ic % KCG),
                                start=(ic == 0),
                                stop=False,
                            )
                        for kp in range(K8_PAIRS):
                            nc.tensor.matmul(
                                psums[oc],
                                lhsT8(kp, oc),
                                rhs8(kp),
                                start=False,
                                stop=False,
                                perf_mode=DR,
                            )
                            ic = NIC - 2 + kp
                            nc.tensor.matmul(
                                psums[oc],
                                lhsT(ic, oc),
                                rhs(ic // KCG, ic % KCG),
                                start=False,
                                stop=(kp == K8_PAIRS - 1),
                            )

                    if last:
                        # oc-major so each psum finishes early and its
                        # drain + output DMA overlap remaining matmuls
                        for oc in range(OT):
                            emit_oc_mms(oc)
                            drain(psums[oc], oc, ch, h)
                        continue
                    prefetch = []
                    for g in range(KGB):
                        kk = KCG if g < KGB - 1 else KCG - 2
                        for k in range(kk):
                            ic = g * KCG + k
                            for oc in range(OT):
                                nc.tensor.matmul(
                                    psums[oc],
                                    lhsT(ic, oc),
                                    rhs(g, k),
                                    start=(ic == 0),
                                    stop=False,
                                )
                        if h == 1 and ch + 1 < N_CHUNK and g < KGB - 1:
                            # spread next-chunk prefetch through this half
                            prefetch.append(emit_x_group(ch + 1, g))
                    # tail of the half: DR,bf16 alternation (DR LDWEIGHTS
                    # hides under the neighboring bf16 matmul), stop on
                    # the final bf16, drain right after each oc's stop.
                    for oc in range(OT):
                        nc.tensor.matmul(
                            psums[oc],
                            lhsT8(0, oc),
                            rhs8(0),
                            start=False,
                            stop=False,
                            perf_mode=DR,
                        )
                        nc.tensor.matmul(
                            psums[oc],
                            lhsT(NIC - 2, oc),
                            rhs(KGB - 1, KCG - 2),
                            start=False,
                            stop=False,
                        )
                    for oc in range(OT):
                        nc.tensor.matmul(
                            psums[oc],
                            lhsT8(1, oc),
                            rhs8(1),
                            start=False,
                            stop=False,
                            perf_mode=DR,
                        )
                        nc.tensor.matmul(
                            psums[oc],
                            lhsT(NIC - 1, oc),
                            rhs(KGB - 1, KCG - 1),
                            start=False,
                            stop=True,
                        )
                        drain(psums[oc], oc, ch, h)
                    if h == 1 and ch + 1 < N_CHUNK:
                        prefetch.append(emit_x_group(ch + 1, KGB - 1))
                        prefetch.append(emit_x8(ch + 1))
                        pending = prefetch

    nc.compile()
    return nc


_NC_CACHE = None


def _shard_inputs(x, weight, mask):
    """Host-side marshalling: premultiply mask, scale by 32, transpose,
    split the contraction dim into bf16 and fp8 parts, pre-pack into the
    exact SBUF tile layouts, slice per core."""
    x = np.asarray(x, dtype=np.float32)
    weight = np.asarray(weight, dtype=np.float32)
    mask = np.asarray(mask, dtype=np.float32)
    xT = x.T
    xT_b = xT[:KB].astype(ml_dtypes.bfloat16)
    xT_8 = xT[KB:].astype(ml_dtypes.float8_e4m3)

    # xTp [NC][KGB][P][KCG][1024]: row (g*KCG*P + kc*P + p) -> [g][p][kc]
    xb = xT_b.reshape(KGB, KCG, P, N_CHUNK, BC_DMA)
    xTp = np.ascontiguousarray(xb.transpose(3, 0, 2, 1, 4)).reshape(N_CHUNK * KGB * P, KCG, BC_DMA)
    # x0Tp [2][KGB][P][KCG][512]: chunk 0 split into batch halves
    x0 = xT_b[:, 0:BC_DMA].reshape(KGB, KCG, P, 2, BN)
    x0Tp = np.ascontiguousarray(x0.transpose(3, 0, 2, 1, 4)).reshape(2 * KGB * P, KCG, BN)
    # x8Tp [NC][P][KP][2][1024]: row (kp*2*P + ko*P + p) -> [p][kp][ko]
    x8 = xT_8.reshape(K8_PAIRS, 2, P, N_CHUNK, BC_DMA)
    x8Tp = np.ascontiguousarray(x8.transpose(3, 2, 0, 1, 4)).reshape(N_CHUNK * P, K8_PAIRS, 2, BC_DMA)
    x80 = xT_8[:, 0:BC_DMA].reshape(K8_PAIRS, 2, P, 2, BN)
    x80Tp = np.ascontiguousarray(x80.transpose(3, 2, 0, 1, 4)).reshape(2 * P, K8_PAIRS, 2, BN)

    wsT = ((weight * mask) * np.float32(WSCALE)).T
    in_maps = []
    for c in range(N_CORES):
        sl = slice(c * O_PER_CORE, (c + 1) * O_PER_CORE)
        wb = wsT[:KB, sl].astype(ml_dtypes.bfloat16)
        # wmTp [KGB][P][KCG][512]
        wmTp = np.ascontiguousarray(
            wb.reshape(KGB, KCG, P, O_PER_CORE).transpose(0, 2, 1, 3)
        ).reshape(KGB * P, KCG, O_PER_CORE)
        w8 = wsT[KB:, sl].astype(ml_dtypes.float8_e4m3)
        # wm8Tp [P][KP][2][512]
        wm8Tp = np.ascontiguousarray(
            w8.reshape(K8_PAIRS, 2, P, O_PER_CORE).transpose(2, 0, 1, 3)
        )
        in_maps.append(
            {
                "xTp": xTp,
                "x0Tp": x0Tp,
                "x8Tp": x8Tp,
                "x80Tp": x80Tp,
                "wmTp": wmTp,
                "wm8Tp": wm8Tp,
            }
        )
    return in_maps


def kernel(x, weight, mask):
    global _NC_CACHE
    if _NC_CACHE is None:
        _NC_CACHE = build_nc()
    nc = _NC_CACHE

    in_maps = _shard_inputs(x, weight, mask)
    res = run_bass_kernel_spmd(nc, in_maps, core_ids=list(range(N_CORES)))

    out = np.empty((BATCH, D_OUT), dtype=np.float32)
    for c in range(N_CORES):
        sl = slice(c * O_PER_CORE, (c + 1) * O_PER_CORE)
        # outTp [OT][NC][2][P][BN] -> out[ch*1024 + h*512 + b, oc*128 + p]
        r = res.results[c]["outTp"].reshape(OT, N_CHUNK, 2, P, BN)
        out[:, sl] = r.transpose(1, 2, 4, 0, 3).reshape(BATCH, O_PER_CORE)
    return out


# revision 14
# speedup vs baseline: 1.1980x; 1.1980x over previous
"""Trainium2 Bass kernel for ExpanderLinear: out = x @ (W * mask).T

Shapes (hardcoded): x [8192, 4096] f32, weight [4096, 4096] f32,
mask [4096, 4096] f32 -> out [8192, 4096] f32.

Strategy: tensor-parallel over output features across 8 cores. The host
pre-marshals operands (like GEMM pre-packing): wm = (W*mask)*32
premultiplied, transposed, and split along the contraction dim:
  - rows 0..3583  -> bf16   (28 of 32 contraction chunks)
  - rows 3584..4095 -> fp8e4m3, computed with DoubleRow matmuls
    (2 contraction chunks of 256 per instruction, 2 MACs/cell/cycle)
x is transposed and split the same way (bf16 + fp8). The *32 weight
scale (exact in bf16, keeps fp8 weights out of the subnormal range) is
undone by the PSUM-drain copy (tensor_scalar_mul 1/32). Measured absmax
error vs the f64 reference: 1.80e-2 of scale (tolerance 2e-2); the
bf16-only variant measures 2.45e-3.

All host arrays are pre-packed into the exact SBUF tile layouts so
every DMA moves one fully contiguous block (no strided gathers):
  xTp   [NC][KGB][P][KCG][1024] bf16   (chunks 1..7; slot 0 unused)
  x0Tp  [2][KGB][P][KCG][512]   bf16   (chunk 0, per batch half)
  x8Tp  [NC][P][KP][2][1024]    fp8    (chunks 1..7)
  x80Tp [2][P][KP][2][512]      fp8    (chunk 0)
  wmTp  [KGB][P][KCG][512]      bf16
  wm8Tp [P][KP][2][512]         fp8
  outTp [OT][NC][2][P][512]     f32    (drain DMA writes contiguous)

Per-core device kernel:
  - PE warmup matmuls on a memset tile run during the initial DMA wait
    so the HAM clock gate is at 2.4 GHz when data lands.
  - weights persist in SBUF (3.5 MB bf16 + 0.25 MB fp8), loaded
    interleaved with the first chunk's x loads (chunk 0 uses 512-col
    half tiles so the first matmul's dependencies are only ~1 MB).
  - loop over 8 batch chunks of 1024, each split into two 512-wide
    matmul halves: 4 psum banks per half; per oc 28 bf16 matmuls + 2
    fp8 DoubleRow matmuls accumulate. The 2 DR matmuls are interleaved
    with the last bf16 matmuls (DR,bf16 alternation) so the DR
    LDWEIGHTS (non-FWL, ~107 ns) hides under a 213 ns bf16 matmul
    instead of serializing. Each oc's drain is emitted right after its
    stop matmul so the psum bank frees early. Prefetch for chunk c+1 is
    spread through c's second half for ~25 us of DMA lead time.
"""

import ml_dtypes
import numpy as np

import concourse.bass as bass
import concourse.mybir as mybir
import concourse.tile as tile
from concourse import bacc
from concourse.bass_utils import run_bass_kernel_spmd

P = 128
D_IN = 4096
D_OUT = 4096
BATCH = 8192
N_CORES = 8
O_PER_CORE = D_OUT // N_CORES  # 512
OT = O_PER_CORE // P  # 4 output partition tiles
KCG = 4  # contraction chunks (of 128) per bf16 group
K8_PAIRS = 2  # fp8 DoubleRow matmuls per oc (each covers 256 of K)
K8 = K8_PAIRS * 2 * P  # 512 contraction rows in fp8
KB = D_IN - K8  # 3584 contraction rows in bf16
KGB = KB // (KCG * P)  # 7 bf16 groups
NIC = KB // P  # 28 bf16 contraction chunks
BC_DMA = 1024  # batch columns per DMA tile (2 KB bf16 lines)
N_CHUNK = BATCH // BC_DMA  # 8
BN = 512  # matmul free dim (one psum bank)
N_WARM = 56  # PE warmup matmuls (N=128)
WSCALE = 32.0  # host weight pre-scale, undone in the drain

F32 = mybir.dt.float32
BF16 = mybir.dt.bfloat16
F8 = mybir.dt.float8e4
DR = mybir.MatmulPerfMode.DoubleRow


def build_nc():
    nc = bacc.Bacc("TRN2", target_bir_lowering=False, debug=False, num_devices=N_CORES)

    xT_d = nc.dram_tensor("xTp", [N_CHUNK * KGB * P, KCG, BC_DMA], BF16, kind="ExternalInput")
    x0T_d = nc.dram_tensor("x0Tp", [2 * KGB * P, KCG, BN], BF16, kind="ExternalInput")
    x8T_d = nc.dram_tensor("x8Tp", [N_CHUNK * P, K8_PAIRS, 2, BC_DMA], F8, kind="ExternalInput")
    x80T_d = nc.dram_tensor("x80Tp", [2 * P, K8_PAIRS, 2, BN], F8, kind="ExternalInput")
    wmT_d = nc.dram_tensor("wmTp", [KGB * P, KCG, O_PER_CORE], BF16, kind="ExternalInput")
    wm8T_d = nc.dram_tensor("wm8Tp", [P, K8_PAIRS, 2, O_PER_CORE], F8, kind="ExternalInput")
    outT_d = nc.dram_tensor("outTp", [OT * N_CHUNK * 2 * P, BN], F32, kind="ExternalOutput")
    outTail_d = nc.dram_tensor("outTailp", [2 * P, 256], F32, kind="ExternalOutput")

    with tile.TileContext(nc) as tc:
        with (
            tc.tile_pool(name="persist", bufs=1) as persist,
            tc.tile_pool(name="xs", bufs=16) as xspool,
            tc.tile_pool(name="outp", bufs=8) as outp,
            tc.tile_pool(name="mpsum", bufs=8, space="PSUM") as mpsum,
        ):
            # --- PE warmup: emitted first so the tensor queue starts on
            # them while the first DMAs are in flight ---
            wtile = persist.tile([P, P], BF16, name="warm_in")
            nc.gpsimd.memset(wtile, 0)
            wpsum = mpsum.tile([P, BN], F32, name="warm_ps", tag="ps")
            for _ in range(N_WARM):
                nc.tensor.matmul(
                    wpsum[:, 0:P], wtile, wtile, start=True, stop=True
                )

            # --- weight loads, interleaved with the first chunk's x ---
            wm_g = []

            def emit_wm_group(g):
                wm = persist.tile([P, KCG, O_PER_CORE], BF16, name=f"wmT{g}")
                nc.sync.dma_start(wm, wmT_d[g * P : (g + 1) * P])
                wm_g.append(wm)

            def emit_x_group(ch, g):
                """bf16 x group tile, full 1024 cols (chunks 1..7)."""
                xs = xspool.tile([P, KCG, BC_DMA], BF16, tag="xs", name="xs")
                r0 = (ch * KGB + g) * P
                nc.sync.dma_start(xs, xT_d[r0 : r0 + P])
                return xs

            def emit_x0_group(h, g):
                """bf16 x group tile for chunk 0, 512-col half."""
                xs = xspool.tile([P, KCG, BN], BF16, tag="xs", name="xs")
                r0 = (h * KGB + g) * P
                nc.sync.dma_start(xs, x0T_d[r0 : r0 + P])
                return xs

            def emit_x8(ch):
                """fp8 x tile [P, pairs, 2, 1024] in DoubleRow pairing."""
                xs = xspool.tile([P, K8_PAIRS, 2, BC_DMA], F8, tag="xs", name="xs8")
                nc.sync.dma_start(xs, x8T_d[ch * P : (ch + 1) * P])
                return xs

            def emit_x80(h):
                xs = xspool.tile([P, K8_PAIRS, 2, BN], F8, tag="xs", name="xs8")
                nc.sync.dma_start(xs, x80T_d[h * P : (h + 1) * P])
                return xs

            x0 = {}
            for g in range(KGB):
                emit_wm_group(g)
                x0[(0, g)] = emit_x0_group(0, g)
            wm8 = persist.tile([P, K8_PAIRS, 2, O_PER_CORE], F8, name="wm8T")
            nc.sync.dma_start(wm8, wm8T_d[0:P])
            x0[(0, KGB)] = emit_x80(0)
            for g in range(KGB):
                x0[(1, g)] = emit_x0_group(1, g)
            x0[(1, KGB)] = emit_x80(1)

            def lhsT(ic, oc):
                return wm_g[ic // KCG][:, ic % KCG, oc * P : (oc + 1) * P]

            def lhsT8(kp, oc):
                return wm8[:, kp, :, oc * P : (oc + 1) * P]

            def drain(psum, oc, ch, h):
                ob = outp.tile([P, BN], F32)
                nc.vector.tensor_scalar_mul(ob, psum, 1.0 / WSCALE)
                r0 = ((oc * N_CHUNK + ch) * 2 + h) * P
                nc.sync.dma_start(outT_d[r0 : r0 + P], ob)

            # --- main loop over batch chunks ---
            pending = None
            for ch in range(N_CHUNK):
                xs_g = pending
                for h in range(2):
                    def rhs(g, k):
                        if ch == 0:
                            return x0[(h, g)][:, k, :]
                        return xs_g[g][:, k, h * BN : (h + 1) * BN]

                    def rhs8(kp):
                        if ch == 0:
                            return x0[(h, KGB)][:, kp, :, :]
                        return xs_g[KGB][:, kp, :, h * BN : (h + 1) * BN]

                    last = ch == N_CHUNK - 1 and h == 1
                    psums = [
                        mpsum.tile([P, BN], F32, name=f"ps{oc}", tag="ps")
                        for oc in range(OT)
                    ]

                    def emit_oc_mms(oc):
                        # oc-major (last half): DR matmuls interleaved
                        # between bf16 ones so their LDWEIGHTS hide.
                        for ic in range(NIC - 2):
                            nc.tensor.matmul(
                                psums[oc],
                                lhsT(ic, oc),
                                rhs(ic // KCG, ic % KCG),
                                start=(ic == 0),
                                stop=False,
                            )
                        for kp in range(K8_PAIRS):
                            nc.tensor.matmul(
                                psums[oc],
                                lhsT8(kp, oc),
                                rhs8(kp),
                                start=False,
                                stop=False,
                                perf_mode=DR,
                            )
                            ic = NIC - 2 + kp
                            nc.tensor.matmul(
                                psums[oc],
                                lhsT(ic, oc),
                                rhs(ic // KCG, ic % KCG),
                                start=False,
                                stop=(kp == K8_PAIRS - 1),
                            )

                    if last:
                        # oc-major so each psum finishes early and its
                        # drain + output DMA overlap remaining matmuls;
                        # the final oc runs as two 256-col groups so the
                        # very last drain + DMA are half-sized.
                        for oc in range(OT - 1):
                            emit_oc_mms(oc)
                            drain(psums[oc], oc, ch, h)
                        for q in range(2):
                            c0 = h * BN + q * 256
                            psq = mpsum.tile([P, 256], F32, name=f"psq{q}", tag="ps")
                            for ic in range(NIC - 2):
                                nc.tensor.matmul(
                                    psq,
                                    lhsT(ic, OT - 1),
                                    xs_g[ic // KCG][:, ic % KCG, c0 : c0 + 256],
                                    start=(ic == 0),
                                    stop=False,
                                )
                            for kp in range(K8_PAIRS):
                                nc.tensor.matmul(
                                    psq,
                                    lhsT8(kp, OT - 1),
                                    xs_g[KGB][:, kp, :, c0 : c0 + 256],
                                    start=False,
                                    stop=False,
                                    perf_mode=DR,
                                )
                                ic = NIC - 2 + kp
                                nc.tensor.matmul(
                                    psq,
                                    lhsT(ic, OT - 1),
                                    xs_g[ic // KCG][:, ic % KCG, c0 : c0 + 256],
                                    start=False,
                                    stop=(kp == K8_PAIRS - 1),
                                )
                            obq = outp.tile([P, 256], F32)
                            nc.vector.tensor_scalar_mul(obq, psq, 1.0 / WSCALE)
                            nc.sync.dma_start(outTail_d[q * P : (q + 1) * P], obq)
                        continue
                    prefetch = []
                    for g in range(KGB):
                        kk = KCG if g < KGB - 1 else KCG - 2
                        for k in range(kk):
                            ic = g * KCG + k
                            for oc in range(OT):
                                nc.tensor.matmul(
                                    psums[oc],
                                    lhsT(ic, oc),
                                    rhs(g, k),
                                    start=(ic == 0),
                                    stop=False,
                                )
                        if h == 1 and ch + 1 < N_CHUNK and g < KGB - 1:
                            # spread next-chunk prefetch through this half
                            prefetch.append(emit_x_group(ch + 1, g))
                    # tail of the half: DR,bf16 alternation (DR LDWEIGHTS
                    # hides under the neighboring bf16 matmul), stop on
                    # the final bf16, drain right after each oc's stop.
                    for oc in range(OT):
                        nc.tensor.matmul(
                            psums[oc],
                            lhsT8(0, oc),
                            rhs8(0),
                            start=False,
                            stop=False,
                            perf_mode=DR,
                        )
                        nc.tensor.matmul(
                            psums[oc],
                            lhsT(NIC - 2, oc),
                            rhs(KGB - 1, KCG - 2),
                            start=False,
                            stop=False,
                        )
                    for oc in range(OT):
                        nc.tensor.matmul(
                            psums[oc],
                            lhsT8(1, oc),
                            rhs8(1),
                            start=False,
                            stop=False,
                            perf_mode=DR,
                        )
                        nc.tensor.matmul(
                            psums[oc],
                            lhsT(NIC - 1, oc),
                            rhs(KGB - 1, KCG - 1),
                            start=False,
                            stop=True,
                        )
                        drain(psums[oc], oc, ch, h)
                    if h == 1 and ch + 1 < N_CHUNK:
                        prefetch.append(emit_x_group(ch + 1, KGB - 1))
                        prefetch.append(emit_x8(ch + 1))
                        pending = prefetch

    nc.compile()
    return nc


_NC_CACHE = None


def _shard_inputs(x, weight, mask):
    """Host-side marshalling: premultiply mask, scale by 32, transpose,
    split the contraction dim into bf16 and fp8 parts, pre-pack into the
    exact SBUF tile layouts, slice per core."""
    x = np.asarray(x, dtype=np.float32)
    weight = np.asarray(weight, dtype=np.float32)
    mask = np.asarray(mask, dtype=np.float32)
    xT = x.T
    xT_b = xT[:KB].astype(ml_dtypes.bfloat16)
    xT_8 = xT[KB:].astype(ml_dtypes.float8_e4m3)

    # xTp [NC][KGB][P][KCG][1024]: row (g*KCG*P + kc*P + p) -> [g][p][kc]
    xb = xT_b.reshape(KGB, KCG, P, N_CHUNK, BC_DMA)
    xTp = np.ascontiguousarray(xb.transpose(3, 0, 2, 1, 4)).reshape(N_CHUNK * KGB * P, KCG, BC_DMA)
    # x0Tp [2][KGB][P][KCG][512]: chunk 0 split into batch halves
    x0 = xT_b[:, 0:BC_DMA].reshape(KGB, KCG, P, 2, BN)
    x0Tp = np.ascontiguousarray(x0.transpose(3, 0, 2, 1, 4)).reshape(2 * KGB * P, KCG, BN)
    # x8Tp [NC][P][KP][2][1024]: row (kp*2*P + ko*P + p) -> [p][kp][ko]
    x8 = xT_8.reshape(K8_PAIRS, 2, P, N_CHUNK, BC_DMA)
    x8Tp = np.ascontiguousarray(x8.transpose(3, 2, 0, 1, 4)).reshape(N_CHUNK * P, K8_PAIRS, 2, BC_DMA)
    x80 = xT_8[:, 0:BC_DMA].reshape(K8_PAIRS, 2, P, 2, BN)
    x80Tp = np.ascontiguousarray(x80.transpose(3, 2, 0, 1, 4)).reshape(2 * P, K8_PAIRS, 2, BN)

    wsT = ((weight * mask) * np.float32(WSCALE)).T
    in_maps = []
    for c in range(N_CORES):
        sl = slice(c * O_PER_CORE, (c + 1) * O_PER_CORE)
        wb = wsT[:KB, sl].astype(ml_dtypes.bfloat16)
        # wmTp [KGB][P][KCG][512]
        wmTp = np.ascontiguousarray(
            wb.reshape(KGB, KCG, P, O_PER_CORE).transpose(0, 2, 1, 3)
        ).reshape(KGB * P, KCG, O_PER_CORE)
        w8 = wsT[KB:, sl].astype(ml_dtypes.float8_e4m3)
        # wm8Tp [P][KP][2][512]
        wm8Tp = np.ascontiguousarray(
            w8.reshape(K8_PAIRS, 2, P, O_PER_CORE).transpose(2, 0, 1, 3)
        )
        in_maps.append(
            {
                "xTp": xTp,
                "x0Tp": x0Tp,
                "x8Tp": x8Tp,
                "x80Tp": x80Tp,
                "wmTp": wmTp,
                "wm8Tp": wm8Tp,
            }
        )
    return in_maps


def kernel(x, weight, mask):
    global _NC_CACHE
    if _NC_CACHE is None:
        _NC_CACHE = build_nc()
    nc = _NC_CACHE

    in_maps = _shard_inputs(x, weight, mask)
    res = run_bass_kernel_spmd(nc, in_maps, core_ids=list(range(N_CORES)))

    out = np.empty((BATCH, D_OUT), dtype=np.float32)
    for c in range(N_CORES):
        sl = slice(c * O_PER_CORE, (c + 1) * O_PER_CORE)
        # outTp [OT][NC][2][P][BN] -> out[ch*1024 + h*512 + b, oc*128 + p]
        r = res.results[c]["outTp"].reshape(OT, N_CHUNK, 2, P, BN)
        out[:, sl] = r.transpose(1, 2, 4, 0, 3).reshape(BATCH, O_PER_CORE)
        # final oc of the last half lives in outTailp [2][P][256]
        r2 = res.results[c]["outTailp"].reshape(2, P, 256)
        for q in range(2):
            out[
                (N_CHUNK - 1) * BC_DMA + BN + q * 256 :
                (N_CHUNK - 1) * BC_DMA + BN + (q + 1) * 256,
                c * O_PER_CORE + (OT - 1) * P : (c + 1) * O_PER_CORE,
            ] = r2[q].T
    return out
